# revision 59
# baseline (speedup 1.0000x reference)
"""Trainium2 Bass kernel for nn_PredictionModel (CPC-style prediction scores).

Computation (B=4, L=512, D=512, C=256, K=12, LW=500):
  c_proj[b,l,k,d] = sum_c Wk[k,d,c] * c[b,l,c]          (l < LW)
  zw[b,l,k,d]     = z[b, l+1+k, d]
  pos[b,l,k]      = <c_proj[b,l,k], zw[b,l,k]>
  neg_g[b,n,l,k]  = <c_proj[b,l,k], zw[perm_B[n], perm_L[l], k]>
  neg_len[b,n,l,k]= <c_proj[b,l,k], zw[b, perms_len[n,l], k]>
  out = concat([pos[:,None], neg_g, neg_len], axis=1)   # (B, 9, LW, K)

Sharding: 8 cores = 4 batches x 2 ranges ([0,256) and [244,500); host takes
position <250 from half 0 and >=250 from half 1).

All 9 score sets are computed in d-partition layout: products
P[d, k, l] = cprojT[d, k, l] * window[d, k, l] on DVE/Pool, then the
d-reduction runs on PE as indicator-column matmuls that accumulate every
(group, dchunk) product into ONE PSUM [9, K*LH] scores tile
(row = score set).

Window sources:
 * pos: in-place overlapping AP on resident zT[b] (k,l strides both 1).
 * neg_g: in-place overlapping AP on resident zT[perm_B[n]], evaluated in
   source-row order j with cprojG from host-permuted c (sigma_g =
   argsort(perm_L)); host scatters j->l afterwards (free).
 * neg_len: host-gathered d-part window tiles, DMA-streamed.

cprojT versions (identity + sigma_g) are computed by PE matmuls straight
into [d, k, l] layout chunks and cast to bf16 via ACT copies.
"""

import numpy as np
import ml_dtypes

import bass_rust
import concourse.mybir as mybir
from concourse import bacc
from concourse.tile import TileContext
from concourse import bass_utils

B, L, D, C, K = 4, 512, 512, 256, 12
LW = L - K          # 500
LH = 250            # per-core l (and j) count
L0S = [0, 250]      # absolute start of each half
NM = 2 * B + 1      # 9 score rows per (l, k)
F32 = mybir.dt.float32
BF16 = mybir.dt.bfloat16
BF16_NP = ml_dtypes.bfloat16

NDC = D // 128      # 4 d-part chunks
FREE = K * LH       # 3072 flattened (k, l) columns

_NC = None

# score-row order: 0=pos, 1..4=neg_g (j-order), 5..8=neg_len
# pool_mults: set of (row, dc) chunk-mult assignments run on Pool instead
# of DVE.
CFG = {
    # (row, dc, third) mults run on Pool when (row in pool_rows and
    # third in pool_thirds); everything else on DVE. padd units pre-add
    # dc-pairs of products on DVE, halving their chain matmuls.
    "pool_rows": (1, 2),
    "pool_thirds": (0, 1, 2),
    "padd_rows": (1, 3),
    "padd_thirds": (0, 1, 2),
    "padd_pool": ((1, 0),),
    "pool_last": True,
    "warmup": 6,
    "zw_bufs": 12,
    "prodg_bufs": 24,
    "copy_rot": "a",      # version psum->sbuf copy engine rotation
}


def _win(zt_sb, col, base, nk, nj):
    """Overlapping-window AP [128, nk, nj] over zt_sb[:, col, base:]:
    element (d, k, j) -> zt_sb[d, col, base + k + j]."""
    ap = zt_sb[:, col, base:].copy()
    part = ap.ap[0]
    ap.ap = bass_rust.VecI64Pair([list(part), [1, nk], [1, nj]])
    return ap


def _build_program(cfg=None):
    """One NeuronCore program, identical across the 8 cores."""
    global _NC
    if cfg is None and _NC is not None:
        return _NC
    cfg = {**CFG, **(cfg or {})}
    nc = bacc.Bacc()
    ct_d = nc.dram_tensor("ct", [128, 2, LH], BF16, kind="ExternalInput")
    ctg_d = nc.dram_tensor("ctg", [128, 2, LH], BF16, kind="ExternalInput")
    wkt_d = nc.dram_tensor("wkt", [128, K, 2, D], BF16, kind="ExternalInput")
    # z transposed: slots 0..3 = z[perm_B[n]], slot 4 = z[b] (own batch)
    zt_d = nc.dram_tensor("zt", [NDC, B + 1, 128, L], BF16, kind="ExternalInput")
    # host-gathered d-part windows for neg_len: [q 4, dc 4, d 128, k, l]
    zw_d = nc.dram_tensor("zw", [B, NDC, 128, K, LH], BF16, kind="ExternalInput")
    # all scores: row 0=pos, 1..4=neg_g, 5..8=neg_len; columns (k, l|j)
    outs_d = nc.dram_tensor("outs", [NM, FREE], F32, kind="ExternalOutput")

    pool_rows = set(cfg["pool_rows"])
    pool_thirds = set(cfg["pool_thirds"])
    pool_extra = set(cfg.get("pool_extra", ()))
    NT = cfg.get("nt", 3)     # k-phases
    KT = K // NT              # k's per phase
    NH = KT // 2              # psq kpair tiles per phase

    with TileContext(nc) as tc:
        with (
            tc.tile_pool(name="const", bufs=1) as const_pool,
            tc.tile_pool(name="psum", bufs=cfg.get("psv_bufs", 6), space="PSUM")
            as psum_pool,
            tc.tile_pool(name="psq", bufs=cfg.get("psq_bufs", 2), space="PSUM") as psq_pool,
            tc.tile_pool(name="cpg", bufs=1) as cpg_pool,
            tc.tile_pool(name="zw", bufs=cfg["zw_bufs"]) as zw_pool,
            tc.tile_pool(name="prodg", bufs=cfg["prodg_bufs"]) as prodg_pool,
            tc.tile_pool(name="sgc", bufs=3) as sgc_pool,
        ):
            ct_sb = const_pool.tile([128, 2, LH], BF16, tag="ct", name="ct_sb")
            ctg_sb = const_pool.tile([128, 2, LH], BF16, tag="ctg", name="ctg_sb")
            wkt_sb = const_pool.tile([128, K, 2, D], BF16, tag="wkt", name="wkt_sb")
            # first version kpairs need wkt k 0-3 + ct/ctg only
            nc.sync.dma_start(out=wkt_sb[:, 0:2], in_=wkt_d[:, 0:2])
            nc.sync.dma_start(out=ctg_sb[:], in_=ctg_d[:])
            nc.sync.dma_start(out=ct_sb[:], in_=ct_d[:])
            nc.sync.dma_start(out=wkt_sb[:, 2:4], in_=wkt_d[:, 2:4])

            zt_sb = const_pool.tile(
                [128, NDC * (B + 1), L], BF16, tag="zt", name="zt_sb"
            )

            def load_zt(dc):
                nc.sync.dma_start(
                    out=zt_sb[:, dc * (B + 1) : (dc + 1) * (B + 1)],
                    in_=zt_d[dc].rearrange("s p r -> p s r"),
                )
            # indicator columns: e9s[r][:, r] = 1
            e9s = []
            for r in range(NM):
                t = const_pool.tile([128, NM], BF16, tag=f"e9_{r}", name=f"e9_{r}")
                nc.vector.memset(t[:], 0)
                nc.vector.memset(t[:, r : r + 1], 1.0)
                e9s.append(t)

            # PE pstate warmup during the DMA lead-in: dummy matmuls on a
            # memset scratch tile ramp the tensor engine to full speed
            nwarm = cfg.get("warmup", 0)
            if nwarm:
                wsrc = const_pool.tile([128, 512], BF16, tag="warm", name="warm")
                nc.vector.memset(wsrc[:], 0)
                wps = psum_pool.tile([128, 500], F32, name="wps", tag="ps")
                for i in range(nwarm):
                    nc.tensor.matmul(
                        wps[:], wsrc[:, 0:128], wsrc[:, :500],
                        start=True, stop=True,
                    )

            # ---- cprojT versions: [d 128, k, l] bf16 per dchunk, built
            # just-in-time per k-third inside the main loop ----
            copy_engs = {
                "a": lambda o, i: nc.scalar.copy(o, i),
                "d": lambda o, i: nc.vector.tensor_copy(out=o, in_=i),
                "p": lambda o, i: nc.gpsimd.tensor_copy(out=o, in_=i),
            }
            rot = cfg["copy_rot"]
            cp_n = [0]

            cpi = [
                cpg_pool.tile([128, K, LH], BF16, tag=f"cpi{dc}", name=f"cpi{dc}")
                for dc in range(NDC)
            ]
            cpg = [
                cpg_pool.tile([128, K, LH], BF16, tag=f"cpg{dc}", name=f"cpg{dc}")
                for dc in range(NDC)
            ]

            def build_third(tiles, src_sb, dc, t):
                """Version matmuls + copy for k in [t*KT, (t+1)*KT) of chunk dc."""
                tile = tiles[dc]
                for kp in range(t * KT // 2, (t + 1) * KT // 2):
                    psv = psum_pool.tile(
                        [128, 2, LH], F32, name=f"psv{id(tiles)}_{dc}_{kp}",
                        tag="ps",
                    )
                    for k2 in range(2):
                        for ci in range(2):
                            nc.tensor.matmul(
                                psv[:, k2],
                                wkt_sb[:, kp * 2 + k2, ci,
                                       dc * 128 : (dc + 1) * 128],
                                src_sb[:, ci, :],
                                start=(ci == 0),
                                stop=(ci == 1),
                            )
                    eng = copy_engs[rot[cp_n[0] % len(rot)]]
                    eng(tile[:, kp * 2 : (kp + 1) * 2, :], psv[:])
                    cp_n[0] += 1

            # ---- neg_len window streams, loaded per (t, dc, q) ----
            zw_tiles = {}

            def load_zw(q, dc, t):
                tl = zw_pool.tile(
                    [128, KT, LH], BF16, tag="zw", name=f"zw{q}_{dc}_{t}"
                )
                nc.sync.dma_start(
                    out=tl[:], in_=zw_d[q, dc, :, t * KT : (t + 1) * KT, :]
                )
                zw_tiles[(q, dc, t)] = tl

            # ---- products + indicator-matmul reduction, by k-thirds ----
            # a matmul's output must fit one PSUM bank (512 f32), so each
            # third accumulates into two k-sixth tiles [9, 512] which are
            # drained at the third boundary and recycled. For padd units,
            # dc-pairs of products are pre-added on DVE/Pool so the chain
            # needs half the matmuls.
            padd_rows = set(cfg.get("padd_rows", ()))
            padd_thirds = set(cfg.get("padd_thirds", ()))
            padd_pool = set(cfg.get("padd_pool", ()))  # (row, t) adds on Pool

            padd_extra = set(cfg.get("padd_extra", ()))
            dadd = set(cfg.get("dadd", ()))  # (row, t): pair-add via DMA

            def is_padd(row, t):
                return (row in padd_rows and t in padd_thirds) or (
                    (row, t) in padd_pool or (row, t) in padd_extra
                    or (row, t) in dadd
                )

            psqs = {}
            nmm = {}
            nchain = {}
            stash = {}
            last_mm = []

            def chain_mm(row, t, tile, ap3):
                for h in range(NH):
                    s = NH * t + h
                    nc.tensor.matmul(
                        psqs[s][:],
                        e9s[row][:],
                        ap3[:, h * 2 : (h + 1) * 2, :].rearrange(
                            "p k j -> p (k j)"
                        ),
                        start=(nmm[s] == 0),
                        stop=(nmm[s] == nchain[s] - 1),
                    )
                    nmm[s] += 1

            def unit(row, dc, t, cp, win_t):
                """One (row, dchunk, k-third): mult (+ pair-add) + chain."""
                prodg = prodg_pool.tile(
                    [128, KT, LH], BF16, tag="prodg", name=f"pg{row}_{dc}_{t}"
                )
                last_phase = t == NT - 1 and dc == NDC - 1
                on_pool = ((row in pool_rows and t in pool_thirds)
                           or (row, t) in pool_extra) and (
                    cfg.get("pool_last") or not last_phase)
                eng = nc.gpsimd if on_pool else nc.vector
                eng.tensor_tensor(
                    out=prodg[:],
                    in0=cp[dc][:, t * KT : (t + 1) * KT, :],
                    in1=win_t,
                    op=mybir.AluOpType.mult,
                )
                if not is_padd(row, t):
                    if (t, dc) == (NT - 1, NDC - 1) and cfg.get("split_last"):
                        last_mm.append((row, prodg))
                    else:
                        chain_mm(row, t, prodg, prodg)
                    return
                if dc % 2 == 0:
                    stash[(row, t)] = prodg
                    return
                prev = stash.pop((row, t))
                if (row, t) in dadd:
                    # accumulate via Pool-issued DMA: prev += prodg
                    nc.gpsimd.dma_start(
                        out=prev[:], in_=prodg[:], accum_op=mybir.AluOpType.add
                    )
                    chain_mm(row, t, prev, prev)
                    return
                ssum = prodg_pool.tile(
                    [128, KT, LH], BF16, tag="prodg", name=f"ps{row}_{dc}_{t}"
                )
                aeng = nc.gpsimd if (row, t) in padd_pool else nc.vector
                aeng.tensor_tensor(
                    out=ssum[:], in0=prev[:], in1=prodg[:],
                    op=mybir.AluOpType.add,
                )
                if (t, dc) == (NT - 1, NDC - 1) and cfg.get("split_last"):
                    last_mm.append((row, ssum))
                else:
                    chain_mm(row, t, ssum, ssum)

            # zt slot column index for (slot, dc) in the dc-major zt_sb
            def zslot(slot, dc):
                return dc * (B + 1) + slot

            # DMA order: zt chunks and zw slices interleaved in consumption
            # order (t-outer, dc-inner); remaining wkt chunks after the
            # t0/dc0-dc1 data so the first phases start early
            for dc in range(NDC):
                load_zt(dc)
                for q in range(B):
                    load_zw(q, dc, 0)
                if dc >= 2:
                    nc.sync.dma_start(
                        out=wkt_sb[:, dc * 4 - 4 : dc * 4],
                        in_=wkt_d[:, dc * 4 - 4 : dc * 4],
                    )
            for t in range(1, NT):
                for dc in range(NDC):
                    for q in range(B):
                        load_zw(q, dc, t)

            # main loop: t outer, dc inner; windows per (row, dc, third):
            # in-place APs start at column 1 + t*KT (k offset folds into the
            # window base); streamed tiles are per-third.
            for t in range(NT):
                for h in range(NH):
                    s = NH * t + h
                    psqs[s] = psq_pool.tile(
                        [NM, 2 * LH], F32, name=f"psq{s}", tag="psq"
                    )
                    nmm[s] = 0
                    nchain[s] = sum(
                        NDC // 2 if is_padd(row, t) else NDC
                        for row in range(NM)
                    )
                for dc in range(NDC):
                    build_third(cpi, ct_sb, dc, t)
                    build_third(cpg, ctg_sb, dc, t)
                    rows = [
                        (0, cpi, lambda dc=dc: _win(
                            zt_sb, zslot(B, dc), 1 + t * KT, KT, LH))
                    ] + [
                        (1 + n, cpg, lambda dc=dc, n=n: _win(
                            zt_sb, zslot(n, dc), 1 + t * KT, KT, LH))
                        for n in range(B)
                    ] + [
                        (5 + q, cpi, lambda dc=dc, q=q, t=t: zw_tiles[
                            (q, dc, t)][:])
                        for q in range(B)
                    ]
                    ro = cfg.get("row_order")
                    if ro is not None:
                        rows.sort(key=lambda r: ro.index(r[0]))
                    else:
                        lastp = t == NT - 1 and dc == NDC - 1
                        rows.sort(key=lambda r: (
                            ((r[0] in pool_rows and t in pool_thirds)
                             or (r[0], t) in pool_extra)
                            and (not lastp),
                            # final phase: pool rows first
                            (-1 if (lastp and cfg.get("pool_last")
                                    and r[0] in pool_rows) else 0)))
                    for row, cp, winf in rows:
                        unit(row, dc, t, cp, winf())
                # deferred final-phase matmuls: h-major so sixth h stops
                # (and can drain) before sixth h+1 finishes
                if last_mm and t == NT - 1:
                    for h in range(NH):
                        s = NH * t + h
                        for row, tile in last_mm:
                            nc.tensor.matmul(
                                psqs[s][:],
                                e9s[row][:],
                                tile[:, h * 2 : (h + 1) * 2, :].rearrange(
                                    "p k j -> p (k j)"
                                ),
                                start=(nmm[s] == 0),
                                stop=(nmm[s] == nchain[s] - 1),
                            )
                            nmm[s] += 1
                        sgc = sgc_pool.tile(
                            [NM, 2 * LH], F32, tag="sgc", name=f"sgcL{s}"
                        )
                        nc.scalar.copy(sgc[:], psqs[s][:])
                        nc.sync.dma_start(
                            out=outs_d.rearrange("m (k j) -> m k j", k=K)[
                                :, s * 2 : (s + 1) * 2, :
                            ],
                            in_=sgc[:],
                        )
                    break
                # drain this phase's kpair tiles: PSUM -> SBUF -> DRAM;
                # in the final phase spread copies/DMAs across engines
                for h in range(NH):
                    s = NH * t + h
                    sgc = sgc_pool.tile(
                        [NM, 2 * LH], F32, tag="sgc", name=f"sgc{s}"
                    )
                    sd = cfg.get("split_drain")
                    if t == NT - 1 and h == 1 and sd:
                        nc.vector.tensor_copy(out=sgc[:], in_=psqs[s][:])
                    else:
                        nc.scalar.copy(sgc[:], psqs[s][:])
                    dq = nc.scalar if (t == NT - 1 and h == 1
                                       and sd == 2) else nc.sync
                    dq.dma_start(
                        out=outs_d.rearrange("m (k j) -> m k j", k=K)[
                            :, s * 2 : (s + 1) * 2, :
                        ],
                        in_=sgc[:],
                    )

    nc.compile()
    if cfg == CFG:
        _NC = nc
    return nc


def _make_inputs(c, z, Wk, perms_len, perm_L, perm_B):
    """Host-side sharding: per-core input dicts."""
    z_bf = z.astype(BF16_NP)
    wkt = np.ascontiguousarray(
        Wk.transpose(0, 2, 1).reshape(K, 2, 128, D).transpose(2, 0, 1, 3)
    ).astype(BF16_NP)  # [128, K, 2, D]
    # zT chunks: [NDC, 128, L] per batch
    ztc = np.ascontiguousarray(z_bf.transpose(0, 2, 1).reshape(B, NDC, 128, L))
    perm_B = np.asarray(perm_B, np.int64)
    sigma_g = np.argsort(perm_L).astype(np.int64)  # l = sigma_g[j]
    karr = np.arange(K, dtype=np.int64)
    in_maps = []
    for b in range(B):
        for h in range(2):
            L0 = L0S[h]
            l_abs = np.arange(L0, L0 + LH, dtype=np.int64)

            def ctr(cols):
                return np.ascontiguousarray(
                    c[b, cols, :].T.reshape(2, 128, LH).transpose(1, 0, 2)
                ).astype(BF16_NP)

            ct = ctr(l_abs)
            ctg = ctr(sigma_g[l_abs])
            # zt slots 0..3 = z[perm_B[n]], slot 4 = z[b]; all shifted by L0
            # so window column (1 + j + k) reads z[sb, L0 + j + 1 + k].
            # Layout dc-major: [NDC, B+1, 128, L]
            zt = np.empty((NDC, B + 1, 128, L), dtype=BF16_NP)
            for slot in range(B + 1):
                sb = int(perm_B[slot]) if slot < B else b
                sh = np.zeros((L, 512), dtype=BF16_NP)
                sh[: L - L0] = z_bf[sb, L0:]
                zt[:, slot] = sh.T.reshape(NDC, 128, L)
            # neg_len windows, d-part [q, dc, d, k, l]
            zw = np.empty((B, NDC, 128, K, LH), dtype=BF16_NP)
            for q in range(B):
                sl = perms_len[q, l_abs].astype(np.int64)  # (LH,)
                rows = sl[None, :] + 1 + karr[:, None]     # (K, LH)
                g = z_bf[b, rows]                          # (K, LH, 512)
                zw[q] = g.transpose(2, 0, 1).reshape(NDC, 128, K, LH)
            in_maps.append(
                {"ct": ct, "ctg": ctg, "wkt": wkt, "zw": zw, "zt": zt}
            )
    return in_maps


def kernel(c, z, Wk, perms_len, perm_L, perm_B, _trace=False, _result_holder=None):
    c = np.asarray(c, np.float32)
    z = np.asarray(z, np.float32)
    Wk = np.asarray(Wk, np.float32)
    perms_len = np.asarray(perms_len)
    perm_L = np.asarray(perm_L)
    perm_B = np.asarray(perm_B)

    nc = _build_program()
    in_maps = _make_inputs(c, z, Wk, perms_len, perm_L, perm_B)
    res = bass_utils.run_bass_kernel_spmd(
        nc, in_maps, core_ids=list(range(2 * B)), trace=_trace
    )
    if _result_holder is not None:
        _result_holder.append(res)

    sigma_g = np.argsort(perm_L)
    out = np.empty((B, NM, LW, K), np.float32)
    for b in range(B):
        for h in range(2):
            L0 = L0S[h]
            sc = res.results[2 * b + h]["outs"].reshape(NM, K, LH)
            ii = np.arange(0, LH)
            l_loc = L0 + ii
            # pos + neg_len: identity l-order
            out[b, 0, l_loc, :] = sc[0, :, ii]
            for q in range(B):
                out[b, 1 + B + q, l_loc, :] = sc[5 + q, :, ii]
            # neg_g: j-order scatter
            l_of_j = sigma_g[l_loc]
            for n in range(B):
                out[b, 1 + n, l_of_j, :] = sc[1 + n, :, ii]
    return out
